# revision 11
# baseline (speedup 1.0000x reference)
"""BiAttn kernel for 8 TRN2 NeuronCores.

Math: the additive score e[b,x,y] = (k[b,x]@Wk) + (q[b,y]@Wq) + b is constant
along y up to the sq term, and softmax is shift-invariant, so
    a[b,x,y] = softmax(sq[b,:])[y]   (independent of x)
    out[b,x,h] = sum_y p[b,y] * v[b,y,h] = c[b,h]   for every x.
k and the scalar bias cancel entirely. Per batch: sq = q@Wq, p = exp(sq)/sum,
c = p@v, out = broadcast(c) over X. sq ~ N(0, 0.5) so exp() without
max-subtraction is safe.

Sharding: batch B=8 -> one batch per core, fully data parallel, no
collectives. Per core: read q_b, v_b (16MB f32), write out_b.

Implementation notes:
- Inputs are cast f32->bf16 during the SWDGE input DMA; all on-chip math
  except the f32 reduce/psum accumulation runs bf16 (fp32 matmuls on TRN2
  run two LOW_HIGH passes; bf16 is single-pass). Measured end-to-end rel
  err ~4e-3 vs the 2e-2 gate.
- Chunk sizes shrink toward the end so the post-last-DMA compute tail is
  one tile, not a whole chunk.
"""

import sys

import numpy as np

for _p in ("/opt/trn_rl_repo",):
    if _p not in sys.path:
        sys.path.insert(0, _p)

B, X, Y, H = 8, 2048, 2048, 1024
N_CORES = 8
P = 128            # partitions
NT = Y // P        # 16 y-tiles
CHUNKS = [2, 2, 2, 2, 2, 2, 2, 1, 1]   # y-tiles per input DMA chunk
assert sum(CHUNKS) == NT

OUT_DTYPE = "bfloat16"  # output DRAM dtype; host upcasts to f32
SINGLE_OUT_DMA = True  # one broadcast-source dma_start vs NT plain ones

_cache = {}


def _build():
    import concourse.bass as bass
    import concourse.mybir as mybir
    from concourse import bacc, tile

    f32 = mybir.dt.float32
    bf16 = mybir.dt.bfloat16
    out_dt = getattr(mybir.dt, OUT_DTYPE)

    nc = bacc.Bacc("TRN2", target_bir_lowering=False, debug=False,
                   num_devices=N_CORES, name="biattn")

    q = nc.dram_tensor("q", [Y, H], f32, kind="ExternalInput").ap()
    v = nc.dram_tensor("v", [Y, H], f32, kind="ExternalInput").ap()
    wq = nc.dram_tensor("wq", [P, H], f32, kind="ExternalInput").ap()
    out = nc.dram_tensor("out", [X, H], out_dt, kind="ExternalOutput").ap()

    # per-tile view: tile yt covers rows [yt*128, (yt+1)*128)
    q_t = q.rearrange("(n p) h -> n p h", p=P)
    v_t = v.rearrange("(n p) h -> n p h", p=P)

    with tile.TileContext(nc) as tc:
        with (
            tc.tile_pool(name="const", bufs=1) as constp,
            tc.tile_pool(name="qin", bufs=len(CHUNKS)) as qp,
            tc.tile_pool(name="vin", bufs=len(CHUNKS)) as vp,
            tc.tile_pool(name="scr", bufs=3) as scr,
            tc.tile_pool(name="small", bufs=1) as smallp,
            tc.tile_pool(name="ps_acc", bufs=1, space=bass.MemorySpace.PSUM) as psa,
            tc.tile_pool(name="ps_misc", bufs=2, space=bass.MemorySpace.PSUM) as psm,
        ):
            wq_b = constp.tile([P, H], bf16, tag="wq_b", name="wq_b")
            nc.gpsimd.dma_start(wq_b[:], wq)

            ones_row = constp.tile([1, P], bf16, tag="ones_row", name="ones_row")
            nc.vector.memset(ones_row[:], 1.0)
            ones_col = constp.tile([P, 1], bf16, tag="ones_col", name="ones_col")
            nc.vector.memset(ones_col[:], 1.0)

            sq_all = smallp.tile([P, NT], f32, tag="sq_all", name="sq_all")
            esq_all = smallp.tile([P, NT], bf16, tag="esq_all", name="esq_all")

            ps_c = psa.tile([1, H], f32, tag="ps_c", name="ps_c")
            ps_d = psa.tile([1, 1], f32, tag="ps_d", name="ps_d")

            yt = 0
            for ci, cs in enumerate(CHUNKS):
                q_sb = qp.tile([P, cs * H], bf16, tag="q_sb", name="q_sb",
                               padded_shape=[P, max(CHUNKS) * H])
                nc.gpsimd.dma_start(
                    q_sb[:].rearrange("p (t h) -> p t h", t=cs),
                    q_t[yt:yt + cs].rearrange("n p h -> p n h"))
                v_bf = vp.tile([P, cs * H], bf16, tag="v_bf", name="v_bf",
                               padded_shape=[P, max(CHUNKS) * H])
                nc.gpsimd.dma_start(
                    v_bf[:].rearrange("p (t h) -> p t h", t=cs),
                    v_t[yt:yt + cs].rearrange("n p h -> p n h"))

                sc = scr.tile([P, cs * H], bf16, tag="sc", name="sc",
                              padded_shape=[P, max(CHUNKS) * H])
                nc.vector.tensor_mul(
                    sc[:].rearrange("p (t h) -> p t h", t=cs),
                    q_sb[:].rearrange("p (t h) -> p t h", t=cs),
                    wq_b[:].unsqueeze(1).broadcast_to([P, cs, H]))
                for t in range(cs):
                    dump = scr.tile([P, H], bf16, tag="dump", name="dump")
                    nc.scalar.activation(
                        dump[:], sc[:, t * H:(t + 1) * H],
                        mybir.ActivationFunctionType.Copy,
                        accum_out=sq_all[:, yt:yt + 1])
                    nc.scalar.activation(
                        esq_all[:, yt:yt + 1], sq_all[:, yt:yt + 1],
                        mybir.ActivationFunctionType.Exp)
                    mms = [("d", None), ("c", 0), ("c", 1)] if yt == NT - 1                         else [("c", 0), ("c", 1), ("d", None)]
                    for kind, j in mms:
                        if kind == "c":
                            nc.tensor.matmul(
                                ps_c[:, j * 512:(j + 1) * 512],
                                esq_all[:, yt:yt + 1],
                                v_bf[:, t * H + j * 512:t * H + (j + 1) * 512],
                                start=(yt == 0), stop=(yt == NT - 1))
                        else:
                            nc.tensor.matmul(
                                ps_d[:], esq_all[:, yt:yt + 1], ones_col[:],
                                start=(yt == 0), stop=(yt == NT - 1))
                    yt += 1

            # c = psum_c / d; fold 1/d into the broadcast matmul's ones
            inv_d = smallp.tile([1, 1], f32, tag="inv_d", name="inv_d")
            nc.vector.reciprocal(inv_d[:], ps_d[:])
            ones_sc = smallp.tile([1, P], bf16, tag="ones_sc", name="ones_sc")
            nc.vector.tensor_scalar_mul(ones_sc[:], ones_row[:], inv_d[:])
            c_sb = smallp.tile([1, H], bf16, tag="c_sb", name="c_sb")
            nc.vector.tensor_copy(c_sb[:, 0:512], ps_c[:, 0:512])
            nc.vector.tensor_copy(c_sb[:, 512:H], ps_c[:, 512:H])

            # broadcast c/d to all 128 partitions via K=1 matmul
            bc_sb = smallp.tile([P, H], out_dt, tag="bc_sb", name="bc_sb")
            for j in range(2):
                ps_b = psm.tile([P, 512], f32, tag="ps_b", name="ps_b")
                nc.tensor.matmul(ps_b[:], ones_sc[:],
                                 c_sb[:, j * 512:(j + 1) * 512],
                                 start=True, stop=True)
                nc.scalar.copy(bc_sb[:, j * 512:(j + 1) * 512], ps_b[:])

            if SINGLE_OUT_DMA:
                dest = out.rearrange("(t p) h -> p t h", p=P)
                src = bc_sb[:].unsqueeze(1).broadcast_to([P, NT, H])
                nc.sync.dma_start(dest, src)
            else:
                out_r = out.rearrange("(t p) h -> t p h", p=P)
                for t in range(NT):
                    nc.sync.dma_start(out_r[t], bc_sb[:])
    nc.compile()
    return nc


def _get_nc():
    if "nc" not in _cache:
        _cache["nc"] = _build()
    return _cache["nc"]


def _in_maps(q, k, v, W, b):
    q = np.asarray(q, dtype=np.float32)
    v = np.asarray(v, dtype=np.float32)
    W = np.asarray(W, dtype=np.float32)
    wq = np.ascontiguousarray(np.broadcast_to(W[H:], (P, H)))
    return [
        {"q": np.ascontiguousarray(q[c]),
         "v": np.ascontiguousarray(v[c]),
         "wq": wq}
        for c in range(N_CORES)
    ]


def kernel(q, k, v, W, b):
    from concourse.bass_utils import run_bass_kernel_spmd

    nc = _get_nc()
    res = run_bass_kernel_spmd(nc, _in_maps(q, k, v, W, b),
                               core_ids=list(range(N_CORES)))
    outs = [np.asarray(res.results[c]["out"]).astype(np.float32)
            for c in range(N_CORES)]
    return np.stack(outs)


# revision 12
# speedup vs baseline: 1.1203x; 1.1203x over previous
"""BiAttn kernel for 8 TRN2 NeuronCores.

Math: the additive score e[b,x,y] = (k[b,x]@Wk) + (q[b,y]@Wq) + b is constant
along y up to the sq term, and softmax is shift-invariant, so
    a[b,x,y] = softmax(sq[b,:])[y]   (independent of x)
    out[b,x,h] = sum_y p[b,y] * v[b,y,h] = c[b,h]   for every x.
k and the scalar bias cancel entirely. Per batch: sq = q@Wq, p = exp(sq)/sum,
c = p@v, out = broadcast(c) over X. sq ~ N(0, 0.5) so exp() without
max-subtraction is safe.

Sharding: batch B=8 -> one batch per core, fully data parallel, no
collectives. Per core: read q_b, v_b (16MB f32), write out_b.

Implementation notes:
- Inputs are cast f32->bf16 during the SWDGE input DMA; all on-chip math
  except the f32 reduce/psum accumulation runs bf16 (fp32 matmuls on TRN2
  run two LOW_HIGH passes; bf16 is single-pass). Measured end-to-end rel
  err ~4e-3 vs the 2e-2 gate.
- Chunk sizes shrink toward the end so the post-last-DMA compute tail is
  one tile, not a whole chunk.
"""

import sys

import numpy as np

for _p in ("/opt/trn_rl_repo",):
    if _p not in sys.path:
        sys.path.insert(0, _p)

B, X, Y, H = 8, 2048, 2048, 1024
N_CORES = 8
P = 128            # partitions
NT = Y // P        # 16 y-tiles
CHUNKS = [2, 2, 2, 2, 2, 2, 2, 1, 1]   # y-tiles per input DMA chunk
assert sum(CHUNKS) == NT

OUT_DTYPE = "bfloat16"  # output DRAM dtype; host upcasts to f32
SINGLE_OUT_DMA = False  # one broadcast-source dma_start vs NT plain ones

_cache = {}


def _build():
    import concourse.bass as bass
    import concourse.mybir as mybir
    from concourse import bacc, tile

    f32 = mybir.dt.float32
    bf16 = mybir.dt.bfloat16
    out_dt = getattr(mybir.dt, OUT_DTYPE)

    nc = bacc.Bacc("TRN2", target_bir_lowering=False, debug=False,
                   num_devices=N_CORES, name="biattn")

    q = nc.dram_tensor("q", [Y, H], f32, kind="ExternalInput").ap()
    v = nc.dram_tensor("v", [Y, H], f32, kind="ExternalInput").ap()
    wq = nc.dram_tensor("wq", [P, H], f32, kind="ExternalInput").ap()
    out = nc.dram_tensor("out", [X, H], out_dt, kind="ExternalOutput").ap()

    # per-tile view: tile yt covers rows [yt*128, (yt+1)*128)
    q_t = q.rearrange("(n p) h -> n p h", p=P)
    v_t = v.rearrange("(n p) h -> n p h", p=P)

    with tile.TileContext(nc) as tc:
        with (
            tc.tile_pool(name="const", bufs=1) as constp,
            tc.tile_pool(name="qin", bufs=len(CHUNKS)) as qp,
            tc.tile_pool(name="vin", bufs=len(CHUNKS)) as vp,
            tc.tile_pool(name="scr", bufs=3) as scr,
            tc.tile_pool(name="small", bufs=1) as smallp,
            tc.tile_pool(name="ps_acc", bufs=1, space=bass.MemorySpace.PSUM) as psa,
            tc.tile_pool(name="ps_misc", bufs=2, space=bass.MemorySpace.PSUM) as psm,
        ):
            wq_b = constp.tile([P, H], bf16, tag="wq_b", name="wq_b")
            nc.gpsimd.dma_start(wq_b[:], wq)

            ones_row = constp.tile([1, P], bf16, tag="ones_row", name="ones_row")
            nc.vector.memset(ones_row[:], 1.0)
            ones_col = constp.tile([P, 1], bf16, tag="ones_col", name="ones_col")
            nc.vector.memset(ones_col[:], 1.0)

            sq_all = smallp.tile([P, NT], f32, tag="sq_all", name="sq_all")
            esq_all = smallp.tile([P, NT], bf16, tag="esq_all", name="esq_all")

            ps_c = psa.tile([1, H], f32, tag="ps_c", name="ps_c")
            ps_d = psa.tile([1, 1], f32, tag="ps_d", name="ps_d")

            yt = 0
            for ci, cs in enumerate(CHUNKS):
                q_sb = qp.tile([P, cs * H], bf16, tag="q_sb", name="q_sb",
                               padded_shape=[P, max(CHUNKS) * H])
                nc.gpsimd.dma_start(
                    q_sb[:].rearrange("p (t h) -> p t h", t=cs),
                    q_t[yt:yt + cs].rearrange("n p h -> p n h"))
                v_bf = vp.tile([P, cs * H], bf16, tag="v_bf", name="v_bf",
                               padded_shape=[P, max(CHUNKS) * H])
                nc.gpsimd.dma_start(
                    v_bf[:].rearrange("p (t h) -> p t h", t=cs),
                    v_t[yt:yt + cs].rearrange("n p h -> p n h"))

                sc = scr.tile([P, cs * H], bf16, tag="sc", name="sc",
                              padded_shape=[P, max(CHUNKS) * H])
                nc.vector.tensor_mul(
                    sc[:].rearrange("p (t h) -> p t h", t=cs),
                    q_sb[:].rearrange("p (t h) -> p t h", t=cs),
                    wq_b[:].unsqueeze(1).broadcast_to([P, cs, H]))
                for t in range(cs):
                    dump = scr.tile([P, H], bf16, tag="dump", name="dump")
                    nc.scalar.activation(
                        dump[:], sc[:, t * H:(t + 1) * H],
                        mybir.ActivationFunctionType.Copy,
                        accum_out=sq_all[:, yt:yt + 1])
                    nc.scalar.activation(
                        esq_all[:, yt:yt + 1], sq_all[:, yt:yt + 1],
                        mybir.ActivationFunctionType.Exp)
                    mms = [("d", None), ("c", 0), ("c", 1)] if yt == NT - 1                         else [("c", 0), ("c", 1), ("d", None)]
                    for kind, j in mms:
                        if kind == "c":
                            nc.tensor.matmul(
                                ps_c[:, j * 512:(j + 1) * 512],
                                esq_all[:, yt:yt + 1],
                                v_bf[:, t * H + j * 512:t * H + (j + 1) * 512],
                                start=(yt == 0), stop=(yt == NT - 1))
                        else:
                            nc.tensor.matmul(
                                ps_d[:], esq_all[:, yt:yt + 1], ones_col[:],
                                start=(yt == 0), stop=(yt == NT - 1))
                    yt += 1

            # c = psum_c / d; fold 1/d into the broadcast matmul's ones
            inv_d = smallp.tile([1, 1], f32, tag="inv_d", name="inv_d")
            nc.vector.reciprocal(inv_d[:], ps_d[:])
            ones_sc = smallp.tile([1, P], bf16, tag="ones_sc", name="ones_sc")
            nc.vector.tensor_scalar_mul(ones_sc[:], ones_row[:], inv_d[:])
            c_sb = smallp.tile([1, H], bf16, tag="c_sb", name="c_sb")
            nc.vector.tensor_copy(c_sb[:, 0:512], ps_c[:, 0:512])
            nc.vector.tensor_copy(c_sb[:, 512:H], ps_c[:, 512:H])

            # broadcast c/d to all 128 partitions via K=1 matmul
            bc_sb = smallp.tile([P, H], out_dt, tag="bc_sb", name="bc_sb")
            for j in range(2):
                ps_b = psm.tile([P, 512], f32, tag="ps_b", name="ps_b")
                nc.tensor.matmul(ps_b[:], ones_sc[:],
                                 c_sb[:, j * 512:(j + 1) * 512],
                                 start=True, stop=True)
                nc.scalar.copy(bc_sb[:, j * 512:(j + 1) * 512], ps_b[:])

            if SINGLE_OUT_DMA:
                dest = out.rearrange("(t p) h -> p t h", p=P)
                src = bc_sb[:].unsqueeze(1).broadcast_to([P, NT, H])
                nc.sync.dma_start(dest, src)
            else:
                out_r = out.rearrange("(t p) h -> t p h", p=P)
                for t in range(NT):
                    nc.sync.dma_start(out_r[t], bc_sb[:])
    nc.compile()
    return nc


def _get_nc():
    if "nc" not in _cache:
        _cache["nc"] = _build()
    return _cache["nc"]


def _in_maps(q, k, v, W, b):
    q = np.asarray(q, dtype=np.float32)
    v = np.asarray(v, dtype=np.float32)
    W = np.asarray(W, dtype=np.float32)
    wq = np.ascontiguousarray(np.broadcast_to(W[H:], (P, H)))
    return [
        {"q": np.ascontiguousarray(q[c]),
         "v": np.ascontiguousarray(v[c]),
         "wq": wq}
        for c in range(N_CORES)
    ]


def kernel(q, k, v, W, b):
    from concourse.bass_utils import run_bass_kernel_spmd

    nc = _get_nc()
    res = run_bass_kernel_spmd(nc, _in_maps(q, k, v, W, b),
                               core_ids=list(range(N_CORES)))
    outs = [np.asarray(res.results[c]["out"]).astype(np.float32)
            for c in range(N_CORES)]
    return np.stack(outs)
